# revision 6
# baseline (speedup 1.0000x reference)
"""Paged-attention decode kernel for Trainium2 (8 NeuronCores, SPMD).

Strategy (data-parallel over sequences, per the sharding hint):
  - 64 sequences are sorted by context length (desc) and dealt round-robin
    to 8 cores (slot j on core c = sorted rank 8*j + c), so all cores see
    a similar work profile and one shared program (subtile counts baked
    per slot = max over the cores' rank group) wastes little.
  - Host pre-work: scatter the new k/v token into the gathered per-seq
    2048-token windows (via block table), convert to bf16, and lay K/V
    out d-major / token-major so each slot loads with a few large fully
    contiguous DMAs and the device never transposes anything.
  - All matmul operands are bf16 (fp32 stationary loads run at 1/4 rate
    on the PE and dominated the old fp32 kernel); PSUM accumulation stays
    fp32, softmax exp stays fp32-in/bf16-out.
  - Device per (slot, 128-token subtile):
      scores[s,4h..] = K_tile^T.T @ qT        (PE, bf16 in, f32 PSUM)
      E = exp(scores * scale + mask_bias)     (ACT; bias=-1e30 on pads)
      outT[d,4h..]  += V_tile.T @ E_head      (PE accumulate in PSUM)
      den[1,32]     += ones.T @ E             (PE accumulate in PSUM)
    The softmax max-subtraction is skipped: scores*scale ~ N(0,1) for
    this problem, exp stays comfortably in fp32 range.
  - Host post-work: out = (outT / den).T per (core, slot), reordered to
    the original batch order.
"""

import os
import sys
from math import ceil

import numpy as np

# Problem constants (hardcoded per harness contract).
NUM_HEADS = 32
KVH = 8            # kv heads
D = 128            # head dim
NUM_BLOCKS = 512
BS = 256           # block size
MAX_BLOCKS = 8
BATCH = 64
MAX_SEQ = MAX_BLOCKS * BS   # 2048
G = NUM_HEADS // KVH        # 4
N_CORES = 8
NSLOT = BATCH // N_CORES    # 8 sequences per core
SUB = 128                   # tokens per subtile
MAXSUB = MAX_SEQ // SUB     # 16
SCALE = 1.0 / np.sqrt(np.float32(D))
NEG = -1e30

_here = [p for p in ("/opt/trn_rl_repo", "/root/.axon_site/_ro/trn_rl_repo") if os.path.isdir(p)]
for _p in _here:
    if _p not in sys.path:
        sys.path.append(_p)


def _ensure_ntff_hook():
    """Install the antenv.axon_hooks NTFF-profile shim if the image lacks it.

    trn_boot.boot() degrades silently when ``antenv.axon_hooks`` is missing,
    but concourse.bass_utils imports it unconditionally when trace=True.
    Recreate the module (and the ctypes hook into libaxon_pjrt.so) so traced
    runs work in this image.
    """
    try:
        from antenv.axon_hooks import get_axon_ntff_profile_hook  # noqa: F401
        return
    except ImportError:
        pass
    import contextlib
    import ctypes
    import types

    try:
        import antenv
    except ImportError:
        return
    mod = types.ModuleType("antenv.axon_hooks")
    _box = [None]
    mod.set_axon_ntff_profile_hook = lambda h: _box.__setitem__(0, h)
    mod.get_axon_ntff_profile_hook = lambda: _box[0]
    sys.modules["antenv.axon_hooks"] = mod
    antenv.axon_hooks = mod

    so_path = "/opt/axon/libaxon_pjrt.so"
    if not os.path.exists(so_path):
        return
    try:
        lib = ctypes.CDLL(so_path)
        if not hasattr(lib, "axon_start_nrt_profile"):
            return
        lib.axon_start_nrt_profile.argtypes = [
            ctypes.POINTER(ctypes.c_int64), ctypes.c_size_t]
        lib.axon_start_nrt_profile.restype = ctypes.c_int64
        lib.axon_stop_nrt_profile.argtypes = [ctypes.c_char_p]
        lib.axon_stop_nrt_profile.restype = ctypes.c_int64

        @contextlib.contextmanager
        def _hook(output_dir, device_ids):
            import jax
            jax.devices()
            if device_ids:
                ids = (ctypes.c_int64 * len(device_ids))(*device_ids)
                rc = lib.axon_start_nrt_profile(ids, len(device_ids))
            else:
                rc = lib.axon_start_nrt_profile(None, 0)
            if rc != 0:
                raise RuntimeError(f"axon_start_nrt_profile rc={rc}")
            try:
                yield
            finally:
                n = lib.axon_stop_nrt_profile(str(output_dir).encode())
                if n < 0:
                    raise RuntimeError(f"axon_stop_nrt_profile rc={n}")

        mod.set_axon_ntff_profile_hook(_hook)
    except Exception:
        pass


_ensure_ntff_hook()

_PROG_CACHE: dict = {}


def _build_program(cnt: tuple, lens: tuple, opts: dict | None = None):
    """Build (and cache) the per-core Bass program for baked subtile counts."""
    opts = dict(opts or {})
    key = (cnt, lens, tuple(sorted(opts.items())))
    if key in _PROG_CACHE:
        return _PROG_CACHE[key]

    import concourse.bass as bass  # noqa: F401
    import concourse.mybir as mybir
    import concourse.tile as tile
    from concourse import bacc

    nc = bacc.Bacc("TRN2", target_bir_lowering=False, debug=False)
    f32 = mybir.dt.float32
    bf16 = mybir.dt.bfloat16
    CH = opts.get("ch", 8)          # subtiles per DMA chunk

    kT = nc.dram_tensor("kT", [NSLOT, D, MAXSUB, KVH, SUB], bf16, kind="ExternalInput").ap()
    v = nc.dram_tensor("v", [NSLOT, SUB, MAXSUB, KVH, D], bf16, kind="ExternalInput").ap()
    # packed tails: only the last subtile's r valid tokens, contiguous per
    # head ([d, h*r + s] layout) so the tail DMA runs at line rate.
    kTt = nc.dram_tensor("kTt", [NSLOT, D, KVH * SUB], bf16, kind="ExternalInput").ap()
    qT = nc.dram_tensor("qT", [D, NSLOT, KVH, G], bf16, kind="ExternalInput").ap()
    mask = nc.dram_tensor("mask", [NSLOT, SUB, MAXSUB], f32, kind="ExternalInput").ap()
    outT = nc.dram_tensor("outT", [NSLOT, D, KVH * G], f32, kind="ExternalOutput").ap()
    den = nc.dram_tensor("den", [NSLOT, 1, KVH * G], f32, kind="ExternalOutput").ap()

    with tile.TileContext(nc) as tc:
        with (
            tc.tile_pool(name="kp", bufs=opts.get("bk", 5)) as kp,
            tc.tile_pool(name="vp", bufs=opts.get("bv", 5)) as vp,
            tc.tile_pool(name="tp", bufs=2) as tpp,
            tc.tile_pool(name="ep", bufs=opts.get("be", 4)) as ep,
            tc.tile_pool(name="sp", bufs=opts.get("bs", 4), space="PSUM") as sp,
            tc.tile_pool(name="op", bufs=2, space="PSUM") as op,
            tc.tile_pool(name="dp", bufs=2, space="PSUM") as dp,
            tc.tile_pool(name="cp", bufs=1) as cp,
            tc.tile_pool(name="os", bufs=2) as osp,
        ):
            # small loads go on the (otherwise idle) gpsimd SWDGE ring so
            # the sync/scalar HWDGE rings start streaming K/V immediately.
            qt_sb = cp.tile([D, NSLOT * KVH * G], bf16, tag="qt")
            nc.gpsimd.dma_start(qt_sb, qT.rearrange("d b h g -> d (b h g)"))
            mask_sb = cp.tile([SUB, NSLOT * MAXSUB], f32, tag="mask")
            for slot in range(NSLOT):
                nc.gpsimd.dma_start(
                    mask_sb[:, slot * MAXSUB:(slot + 1) * MAXSUB], mask[slot])
            ones = cp.tile([SUB, 1], bf16, tag="ones")
            nc.vector.memset(ones, 1.0)

            for slot in range(NSLOT):
                n = cnt[slot]
                L = lens[slot]
                r = L - (n - 1) * SUB  # tokens in last subtile, 1..SUB
                # chunked loads of the n-1 full subtiles: one kt/vt tile per
                # CH subtiles, fully contiguous per partition in the
                # d-major/token-major dram layouts (CH*2KB runs -> near-peak
                # HBM bandwidth). The partial tail subtile loads separately,
                # trimmed to its r valid tokens.
                nf = n - 1
                ktiles, vtiles = [], []
                for c0 in range(0, nf, CH):
                    c1 = min(nf, c0 + CH)
                    w = c1 - c0
                    kt = kp.tile([D, w * KVH * SUB], bf16, tag="kt")
                    nc.sync.dma_start(
                        kt, kT[slot, :, c0:c1].rearrange("d m h s -> d (m h s)"))
                    vt = vp.tile([SUB, w * KVH * D], bf16, tag="vt")
                    nc.scalar.dma_start(
                        vt, v[slot, :, c0:c1].rearrange("s m h d -> s (m h d)"))
                    ktiles.append(kt)
                    vtiles.append(vt)
                kt_tail = tpp.tile([D, KVH * SUB], bf16, tag="ktail")
                nc.sync.dma_start(kt_tail[:, 0:KVH * r], kTt[slot, :, 0:KVH * r])
                vt_tail = tpp.tile([SUB, KVH * D], bf16, tag="vtail")
                nc.scalar.dma_start(
                    vt_tail[0:r], v[slot, 0:r, nf].rearrange("s h d -> s (h d)"))
                acc = op.tile([D, KVH * G], f32, tag="acc")
                dacc = dp.tile([1, KVH * G], f32, tag="dacc")
                for sub in range(n):
                    tail = sub == n - 1
                    m = r if tail else SUB
                    sc = sp.tile([SUB, KVH * G], f32, tag="sc")
                    for h in range(KVH):
                        ksrc = (kt_tail[:, h * r:h * r + m] if tail else
                                ktiles[sub // CH][:, ((sub % CH) * KVH + h) * SUB:
                                                  ((sub % CH) * KVH + h) * SUB + m])
                        nc.tensor.matmul(
                            sc[:m, h * G:(h + 1) * G],
                            ksrc,
                            qt_sb[:, (slot * KVH + h) * G:(slot * KVH + h) * G + G],
                            start=True, stop=True,
                        )
                    et = ep.tile([SUB, KVH * G], bf16, tag="et")
                    nc.scalar.activation(
                        et[:m], sc[:m], mybir.ActivationFunctionType.Exp,
                        bias=mask_sb[:m, slot * MAXSUB + sub: slot * MAXSUB + sub + 1],
                        scale=float(SCALE),
                    )
                    for h in range(KVH):
                        vsrc = (vt_tail[:m, h * D:(h + 1) * D] if tail else
                                vtiles[sub // CH][:m, ((sub % CH) * KVH + h) * D:
                                                  ((sub % CH) * KVH + h) * D + D])
                        # acc is one PSUM bank; start zeroes the whole 2KB
                        # region lazily, so exactly one start/stop per bank.
                        nc.tensor.matmul(
                            acc[:, h * G:(h + 1) * G],
                            vsrc,
                            et[:m, h * G:(h + 1) * G],
                            start=(sub == 0 and h == 0),
                            stop=(sub == n - 1 and h == KVH - 1),
                        )
                    nc.tensor.matmul(dacc, ones[:m], et[:m],
                                     start=(sub == 0), stop=(sub == n - 1))
                o_sb = osp.tile([D, KVH * G], f32, tag="osb")
                nc.vector.tensor_copy(o_sb, acc)
                d_sb = osp.tile([1, KVH * G], f32, tag="dsb")
                nc.vector.tensor_copy(d_sb, dacc)
                nc.gpsimd.dma_start(outT[slot], o_sb)
                nc.gpsimd.dma_start(den[slot], d_sb)

    nc.compile()
    _PROG_CACHE[key] = nc
    return nc


def _host_prep(q, k, v, k_cache, v_cache, block_table, context_lens, slot_mapping):
    """Shard + lay out bf16 inputs for the 8 cores. Vectorized."""
    import ml_dtypes
    bf16 = ml_dtypes.bfloat16

    B = BATCH
    ctx = np.asarray(context_lens).astype(np.int64)
    order = np.argsort(-ctx, kind="stable")  # sorted ranks -> seq index
    lens = tuple(int(ctx[order[N_CORES * j]]) for j in range(NSLOT))
    cnt = tuple(int(ceil(L / SUB)) for L in lens)

    kc = np.asarray(k_cache).reshape(NUM_BLOCKS * BS, KVH, D)
    vc = np.asarray(v_cache).reshape(NUM_BLOCKS * BS, KVH, D)
    k_new = np.asarray(k).reshape(B, KVH, D)
    v_new = np.asarray(v).reshape(B, KVH, D)
    bt = np.asarray(block_table)
    sm = np.asarray(slot_mapping).astype(np.int64)
    qr = np.asarray(q).reshape(B, KVH, G, D)

    pos = np.arange(MAX_SEQ)
    tok = bt[:, pos // BS] * BS + pos % BS        # [B, S] flat slot ids
    k_g = kc[tok]                                  # [B, S, KVH, D] copies
    v_g = vc[tok]
    # overlay scattered new tokens (any sequence's new token whose slot
    # lands in some sequence's window)
    blk, off = sm // BS, sm % BS
    for b2 in range(B):
        rows, cols = np.nonzero(bt == blk[b2])
        for bb, jj in zip(rows, cols):
            p = jj * BS + off[b2]
            k_g[bb, p] = k_new[b2]
            v_g[bb, p] = v_new[b2]

    in_maps = []
    for c in range(N_CORES):
        sel = order[c::N_CORES]                   # slot j -> seq id
        kg4 = k_g[sel].reshape(NSLOT, MAXSUB, SUB, KVH, D)
        vg4 = v_g[sel].reshape(NSLOT, MAXSUB, SUB, KVH, D)
        valid = pos[None, :] < ctx[sel][:, None]  # [NSLOT, S]
        kTt = np.zeros((NSLOT, D, KVH * SUB), bf16)
        for j in range(NSLOT):
            n, L = cnt[j], lens[j]
            r = L - (n - 1) * SUB
            tail = kg4[j, n - 1, :r]              # [r, KVH, D]
            kTt[j, :, :KVH * r] = (
                tail.transpose(2, 1, 0).reshape(D, KVH * r).astype(bf16))
        m = {
            "kT": np.ascontiguousarray(
                kg4.transpose(0, 4, 1, 3, 2)).astype(bf16),   # [j,D,sub,h,s]
            "v": np.ascontiguousarray(
                vg4.transpose(0, 2, 1, 3, 4)).astype(bf16),   # [j,s,sub,h,d]
            "kTt": kTt,
            "qT": np.ascontiguousarray(
                qr[sel].transpose(3, 0, 1, 2)).astype(bf16),  # [D,j,KVH,G]
            "mask": np.where(valid, np.float32(0.0), np.float32(NEG))
                    .reshape(NSLOT, MAXSUB, SUB)
                    .transpose(0, 2, 1).copy(),               # [j,s,sub]
        }
        in_maps.append(m)

    return in_maps, order, cnt, lens


def kernel(q, k, v, k_cache, v_cache, block_table, context_lens, slot_mapping,
           _trace=False, _verbose=False, _opts=None):
    import time as _time

    from concourse.bass_utils import run_bass_kernel_spmd

    _t0 = _time.time()

    def _log(msg):
        if _verbose:
            print(f"[kernel +{_time.time()-_t0:.1f}s] {msg}", flush=True)

    in_maps, order, cnt, lens = _host_prep(
        q, k, v, k_cache, v_cache, block_table, context_lens, slot_mapping)
    _log(f"host prep done, cnt={cnt} lens={lens}")
    nc = _build_program(cnt, lens, opts=_opts)
    _log("program built+compiled")
    res = run_bass_kernel_spmd(nc, in_maps, core_ids=list(range(N_CORES)),
                               trace=_trace)
    _log("device run done")

    out = np.empty((BATCH, NUM_HEADS * D), np.float32)
    for rank in range(BATCH):
        b = int(order[rank])
        c, j = rank % N_CORES, rank // N_CORES
        oT = res.results[c]["outT"][j]          # [D, 32]
        dn = res.results[c]["den"][j]           # [1, 32]
        out[b] = (oT / dn).T.reshape(NUM_HEADS * D)
    if _trace:
        return out, res
    return out


# revision 9
# speedup vs baseline: 1.0615x; 1.0615x over previous
"""Paged-attention decode kernel for Trainium2 (8 NeuronCores, SPMD).

Strategy (data-parallel over sequences, per the sharding hint):
  - 64 sequences are sorted by context length (desc) and dealt round-robin
    to 8 cores (slot j on core c = sorted rank 8*j + c), so all cores see
    a similar work profile and one shared program (subtile counts baked
    per slot = max over the cores' rank group) wastes little.
  - Host pre-work: scatter the new k/v token into the gathered per-seq
    2048-token windows (via block table), convert to bf16, and lay K/V
    out d-major / token-major so each slot loads with a few large fully
    contiguous DMAs and the device never transposes anything.
  - All matmul operands are bf16 (fp32 stationary loads run at 1/4 rate
    on the PE and dominated the old fp32 kernel); PSUM accumulation stays
    fp32, softmax exp stays fp32-in/bf16-out.
  - Device per (slot, 128-token subtile):
      scores[s,4h..] = K_tile^T.T @ qT        (PE, bf16 in, f32 PSUM)
      E = exp(scores * scale + mask_bias)     (ACT; bias=-1e30 on pads)
      outT[d,4h..]  += V_tile.T @ E_head      (PE accumulate in PSUM)
      den[1,32]     += ones.T @ E             (PE accumulate in PSUM)
    The softmax max-subtraction is skipped: scores*scale ~ N(0,1) for
    this problem, exp stays comfortably in fp32 range.
  - Host post-work: out = (outT / den).T per (core, slot), reordered to
    the original batch order.
"""

import os
import sys
from math import ceil

import numpy as np

# Problem constants (hardcoded per harness contract).
NUM_HEADS = 32
KVH = 8            # kv heads
D = 128            # head dim
NUM_BLOCKS = 512
BS = 256           # block size
MAX_BLOCKS = 8
BATCH = 64
MAX_SEQ = MAX_BLOCKS * BS   # 2048
G = NUM_HEADS // KVH        # 4
N_CORES = 8
NSLOT = BATCH // N_CORES    # 8 sequences per core
SUB = 128                   # tokens per subtile
MAXSUB = MAX_SEQ // SUB     # 16
SCALE = 1.0 / np.sqrt(np.float32(D))
NEG = -1e30

_here = [p for p in ("/opt/trn_rl_repo", "/root/.axon_site/_ro/trn_rl_repo") if os.path.isdir(p)]
for _p in _here:
    if _p not in sys.path:
        sys.path.append(_p)


def _ensure_ntff_hook():
    """Install the antenv.axon_hooks NTFF-profile shim if the image lacks it.

    trn_boot.boot() degrades silently when ``antenv.axon_hooks`` is missing,
    but concourse.bass_utils imports it unconditionally when trace=True.
    Recreate the module (and the ctypes hook into libaxon_pjrt.so) so traced
    runs work in this image.
    """
    try:
        from antenv.axon_hooks import get_axon_ntff_profile_hook  # noqa: F401
        return
    except ImportError:
        pass
    import contextlib
    import ctypes
    import types

    try:
        import antenv
    except ImportError:
        return
    mod = types.ModuleType("antenv.axon_hooks")
    _box = [None]
    mod.set_axon_ntff_profile_hook = lambda h: _box.__setitem__(0, h)
    mod.get_axon_ntff_profile_hook = lambda: _box[0]
    sys.modules["antenv.axon_hooks"] = mod
    antenv.axon_hooks = mod

    so_path = "/opt/axon/libaxon_pjrt.so"
    if not os.path.exists(so_path):
        return
    try:
        lib = ctypes.CDLL(so_path)
        if not hasattr(lib, "axon_start_nrt_profile"):
            return
        lib.axon_start_nrt_profile.argtypes = [
            ctypes.POINTER(ctypes.c_int64), ctypes.c_size_t]
        lib.axon_start_nrt_profile.restype = ctypes.c_int64
        lib.axon_stop_nrt_profile.argtypes = [ctypes.c_char_p]
        lib.axon_stop_nrt_profile.restype = ctypes.c_int64

        @contextlib.contextmanager
        def _hook(output_dir, device_ids):
            import jax
            jax.devices()
            if device_ids:
                ids = (ctypes.c_int64 * len(device_ids))(*device_ids)
                rc = lib.axon_start_nrt_profile(ids, len(device_ids))
            else:
                rc = lib.axon_start_nrt_profile(None, 0)
            if rc != 0:
                raise RuntimeError(f"axon_start_nrt_profile rc={rc}")
            try:
                yield
            finally:
                n = lib.axon_stop_nrt_profile(str(output_dir).encode())
                if n < 0:
                    raise RuntimeError(f"axon_stop_nrt_profile rc={n}")

        mod.set_axon_ntff_profile_hook(_hook)
    except Exception:
        pass


_ensure_ntff_hook()

_PROG_CACHE: dict = {}


def _build_program(cnt: tuple, lens: tuple, opts: dict | None = None):
    """Build (and cache) the per-core Bass program for baked subtile counts."""
    opts = dict(opts or {})
    key = (cnt, lens, tuple(sorted(opts.items())))
    if key in _PROG_CACHE:
        return _PROG_CACHE[key]

    import concourse.bass as bass  # noqa: F401
    import concourse.mybir as mybir
    import concourse.tile as tile
    from concourse import bacc

    nc = bacc.Bacc("TRN2", target_bir_lowering=False, debug=False)
    f32 = mybir.dt.float32
    bf16 = mybir.dt.bfloat16
    CH = opts.get("ch", 4)          # subtiles per DMA chunk

    kT = nc.dram_tensor("kT", [NSLOT, D, MAXSUB, KVH, SUB], bf16, kind="ExternalInput").ap()
    v = nc.dram_tensor("v", [NSLOT, SUB, MAXSUB, KVH, D], bf16, kind="ExternalInput").ap()
    # packed tails: only the last subtile's r valid tokens, contiguous per
    # head ([d, h*r + s] layout) so the tail DMA runs at line rate.
    kTt = nc.dram_tensor("kTt", [NSLOT, D, KVH * SUB], bf16, kind="ExternalInput").ap()
    qT = nc.dram_tensor("qT", [D, NSLOT, KVH, G], bf16, kind="ExternalInput").ap()
    mask = nc.dram_tensor("mask", [NSLOT, SUB, MAXSUB], f32, kind="ExternalInput").ap()
    outT = nc.dram_tensor("outT", [D, NSLOT * KVH * G], f32, kind="ExternalOutput").ap()
    den = nc.dram_tensor("den", [1, NSLOT * KVH * G], f32, kind="ExternalOutput").ap()

    def chunk_plan(nf, first):
        """Chunk widths covering nf subtiles; 'first' ramps up small so the
        PE starts as soon as a small first chunk lands."""
        widths = []
        rem, w = nf, 1 if first else CH
        while rem > 0:
            w = min(w, rem)
            widths.append(w)
            rem -= w
            w = CH if widths[-1] >= CH or not first else min(CH, widths[-1] * 3)
        return widths

    with tile.TileContext(nc) as tc:
        with (
            tc.tile_pool(name="kp", bufs=opts.get("bk", 6)) as kp,
            tc.tile_pool(name="vp", bufs=opts.get("bv", 6)) as vp,
            tc.tile_pool(name="tp", bufs=2) as tpp,
            tc.tile_pool(name="ep", bufs=opts.get("be", 4)) as ep,
            tc.tile_pool(name="sp", bufs=opts.get("bs", 4), space="PSUM") as sp,
            tc.tile_pool(name="op", bufs=2, space="PSUM") as op,
            tc.tile_pool(name="dp", bufs=2, space="PSUM") as dp,
            tc.tile_pool(name="cp", bufs=1) as cp,
        ):
            # qT rides the fast sync HWDGE ring ahead of the first K chunk
            # (the first matmul needs it); the mask can trail on gpsimd.
            qt_sb = cp.tile([D, NSLOT * KVH * G], bf16, tag="qt")
            nc.sync.dma_start(qt_sb, qT.rearrange("d b h g -> d (b h g)"))
            mask_sb = cp.tile([SUB, NSLOT * MAXSUB], f32, tag="mask")
            for slot in range(NSLOT):
                nc.gpsimd.dma_start(
                    mask_sb[:, slot * MAXSUB:(slot + 1) * MAXSUB], mask[slot])
            ones = cp.tile([SUB, 1], bf16, tag="ones")
            nc.vector.memset(ones, 1.0)
            o_all = cp.tile([D, NSLOT * KVH * G], f32, tag="oall")
            d_all = cp.tile([1, NSLOT * KVH * G], f32, tag="dall")

            for slot in range(NSLOT):
                n = cnt[slot]
                L = lens[slot]
                r = L - (n - 1) * SUB  # tokens in last subtile, 1..SUB
                # chunked loads of the n-1 full subtiles: one kt/vt tile per
                # chunk, fully contiguous per partition in the d-major/
                # token-major dram layouts (2KB runs -> near-peak HBM
                # bandwidth). The partial tail subtile loads separately,
                # trimmed to its r valid tokens.
                nf = n - 1
                ktiles, vtiles, cof = [], [], []
                c0 = 0
                for w in chunk_plan(nf, slot == 0):
                    c1 = c0 + w
                    kt = kp.tile([D, w * KVH * SUB], bf16, tag="kt")
                    nc.sync.dma_start(
                        kt, kT[slot, :, c0:c1].rearrange("d m h s -> d (m h s)"))
                    vt = vp.tile([SUB, w * KVH * D], bf16, tag="vt")
                    nc.scalar.dma_start(
                        vt, v[slot, :, c0:c1].rearrange("s m h d -> s (m h d)"))
                    ktiles += [kt] * w
                    vtiles += [vt] * w
                    cof += list(range(w))
                    c0 = c1
                kt_tail = tpp.tile([D, KVH * SUB], bf16, tag="ktail")
                nc.sync.dma_start(kt_tail[:, 0:KVH * r], kTt[slot, :, 0:KVH * r])
                vt_tail = tpp.tile([SUB, KVH * D], bf16, tag="vtail")
                nc.scalar.dma_start(
                    vt_tail[0:r], v[slot, 0:r, nf].rearrange("s h d -> s (h d)"))
                acc = op.tile([D, KVH * G], f32, tag="acc")
                dacc = dp.tile([1, KVH * G], f32, tag="dacc")
                for sub in range(n):
                    tail = sub == n - 1
                    m = r if tail else SUB
                    lo = 0 if tail else cof[sub]
                    sc = sp.tile([SUB, KVH * G], f32, tag="sc")
                    for h in range(KVH):
                        ksrc = (kt_tail[:, h * r:h * r + m] if tail else
                                ktiles[sub][:, (lo * KVH + h) * SUB:
                                            (lo * KVH + h) * SUB + m])
                        nc.tensor.matmul(
                            sc[:m, h * G:(h + 1) * G],
                            ksrc,
                            qt_sb[:, (slot * KVH + h) * G:(slot * KVH + h) * G + G],
                            start=True, stop=True,
                        )
                    et = ep.tile([SUB, KVH * G], bf16, tag="et")
                    nc.scalar.activation(
                        et[:m], sc[:m], mybir.ActivationFunctionType.Exp,
                        bias=mask_sb[:m, slot * MAXSUB + sub: slot * MAXSUB + sub + 1],
                        scale=float(SCALE),
                    )
                    for h in range(KVH):
                        vsrc = (vt_tail[:m, h * D:(h + 1) * D] if tail else
                                vtiles[sub][:m, (lo * KVH + h) * D:
                                            (lo * KVH + h) * D + D])
                        # acc is one PSUM bank; start zeroes the whole 2KB
                        # region lazily, so exactly one start/stop per bank.
                        nc.tensor.matmul(
                            acc[:, h * G:(h + 1) * G],
                            vsrc,
                            et[:m, h * G:(h + 1) * G],
                            start=(sub == 0 and h == 0),
                            stop=(sub == n - 1 and h == KVH - 1),
                        )
                    nc.tensor.matmul(dacc, ones[:m], et[:m],
                                     start=(sub == 0), stop=(sub == n - 1))
                nc.vector.tensor_copy(
                    o_all[:, slot * KVH * G:(slot + 1) * KVH * G], acc)
                nc.vector.tensor_copy(
                    d_all[:, slot * KVH * G:(slot + 1) * KVH * G], dacc)
            nc.sync.dma_start(outT, o_all)
            nc.sync.dma_start(den, d_all)

    nc.compile()
    _PROG_CACHE[key] = nc
    return nc


def _host_prep(q, k, v, k_cache, v_cache, block_table, context_lens, slot_mapping):
    """Shard + lay out bf16 inputs for the 8 cores. Vectorized."""
    import ml_dtypes
    bf16 = ml_dtypes.bfloat16

    B = BATCH
    ctx = np.asarray(context_lens).astype(np.int64)
    order = np.argsort(-ctx, kind="stable")  # sorted ranks -> seq index
    lens = tuple(int(ctx[order[N_CORES * j]]) for j in range(NSLOT))
    cnt = tuple(int(ceil(L / SUB)) for L in lens)

    kc = np.asarray(k_cache).reshape(NUM_BLOCKS * BS, KVH, D)
    vc = np.asarray(v_cache).reshape(NUM_BLOCKS * BS, KVH, D)
    k_new = np.asarray(k).reshape(B, KVH, D)
    v_new = np.asarray(v).reshape(B, KVH, D)
    bt = np.asarray(block_table)
    sm = np.asarray(slot_mapping).astype(np.int64)
    qr = np.asarray(q).reshape(B, KVH, G, D)

    pos = np.arange(MAX_SEQ)
    tok = bt[:, pos // BS] * BS + pos % BS        # [B, S] flat slot ids
    k_g = kc[tok]                                  # [B, S, KVH, D] copies
    v_g = vc[tok]
    # overlay scattered new tokens (any sequence's new token whose slot
    # lands in some sequence's window)
    blk, off = sm // BS, sm % BS
    for b2 in range(B):
        rows, cols = np.nonzero(bt == blk[b2])
        for bb, jj in zip(rows, cols):
            p = jj * BS + off[b2]
            k_g[bb, p] = k_new[b2]
            v_g[bb, p] = v_new[b2]

    in_maps = []
    for c in range(N_CORES):
        sel = order[c::N_CORES]                   # slot j -> seq id
        kg4 = k_g[sel].reshape(NSLOT, MAXSUB, SUB, KVH, D)
        vg4 = v_g[sel].reshape(NSLOT, MAXSUB, SUB, KVH, D)
        valid = pos[None, :] < ctx[sel][:, None]  # [NSLOT, S]
        kTt = np.zeros((NSLOT, D, KVH * SUB), bf16)
        for j in range(NSLOT):
            n, L = cnt[j], lens[j]
            r = L - (n - 1) * SUB
            tail = kg4[j, n - 1, :r]              # [r, KVH, D]
            kTt[j, :, :KVH * r] = (
                tail.transpose(2, 1, 0).reshape(D, KVH * r).astype(bf16))
        m = {
            "kT": np.ascontiguousarray(
                kg4.transpose(0, 4, 1, 3, 2)).astype(bf16),   # [j,D,sub,h,s]
            "v": np.ascontiguousarray(
                vg4.transpose(0, 2, 1, 3, 4)).astype(bf16),   # [j,s,sub,h,d]
            "kTt": kTt,
            "qT": np.ascontiguousarray(
                qr[sel].transpose(3, 0, 1, 2)).astype(bf16),  # [D,j,KVH,G]
            "mask": np.where(valid, np.float32(0.0), np.float32(NEG))
                    .reshape(NSLOT, MAXSUB, SUB)
                    .transpose(0, 2, 1).copy(),               # [j,s,sub]
        }
        in_maps.append(m)

    return in_maps, order, cnt, lens


def kernel(q, k, v, k_cache, v_cache, block_table, context_lens, slot_mapping,
           _trace=False, _verbose=False, _opts=None):
    import time as _time

    from concourse.bass_utils import run_bass_kernel_spmd

    _t0 = _time.time()

    def _log(msg):
        if _verbose:
            print(f"[kernel +{_time.time()-_t0:.1f}s] {msg}", flush=True)

    in_maps, order, cnt, lens = _host_prep(
        q, k, v, k_cache, v_cache, block_table, context_lens, slot_mapping)
    _log(f"host prep done, cnt={cnt} lens={lens}")
    nc = _build_program(cnt, lens, opts=_opts)
    _log("program built+compiled")
    res = run_bass_kernel_spmd(nc, in_maps, core_ids=list(range(N_CORES)),
                               trace=_trace)
    _log("device run done")

    out = np.empty((BATCH, NUM_HEADS * D), np.float32)
    for rank in range(BATCH):
        b = int(order[rank])
        c, j = rank % N_CORES, rank // N_CORES
        oT = res.results[c]["outT"][:, j * KVH * G:(j + 1) * KVH * G]  # [D, 32]
        dn = res.results[c]["den"][:, j * KVH * G:(j + 1) * KVH * G]   # [1, 32]
        out[b] = (oT / dn).T.reshape(NUM_HEADS * D)
    if _trace:
        return out, res
    return out
